# revision 10
# baseline (speedup 1.0000x reference)
"""Tensor-parallel MultiHeadAttention (QKV + RoPE + GQA causal SDPA + dense)
for 8 Trainium2 NeuronCores.

Sharding (TP as in TPMultiHeadAttention): core d owns query heads {2d, 2d+1}
and the single kv head d//2 (kv heads replicated across core pairs), plus the
matching 256 columns of the dense projection. Each core produces a full-shape
partial output; the all-reduce is a host-side sum over the 8 partials.

All matmul feeds are fp16 (full PE rate + fast weight load).  The PE's
per-matmul weight reload (~115ns, serial with the stream) is amortized
wherever consecutive matmuls share a stationary operand: the second+
matmuls clear `InstMatmult.ldweights` and are pinned adjacent to their
loader inside a tile_critical block.

Per-core pipeline:
  0. ~14 dummy warmup matmuls on a memset scratch tile flip the PE's HAM
     clock gate to full speed while the first DMAs stream in.
  1. x^T loaded fully resident (8MB fp16); qkv^T = W @ x^T computed in
     f-outer passes (k, v, q0, q1): each weight tile is loaded once and
     streams the four 512-query chunks (ldweights amortized 4x).
  2. RoPE on q,k via a permutation matmul (rotate_half) + DVE combine;
     softmax scale folded into the q-side weights.  v transposed via PE.
  3. S^T[sk, sq] per 128-row sk tile: both heads' scores go into one
     [P, 2, 512] two-bank PSUM pair (one kr weight load, one fused exp).
     Diagonal tiles cover only their visible query range (N in
     {512, 384, 256, 128}); exp(S^T - ln 1024) on ScalarE (bias keeps
     fp16 exp sums small and cancels in normalization); the leading
     128-wide partial triangle is masked multiplicatively after exp.
  4. ctx^T[d, sq] += v_nat.T @ P^T, both heads per vn weight load.
     Denominators: fp16 P^T tiles summed on DVE (2x packed) in two
     parallel chains, column-summed by a ones matmul, reciprocal'd, then
     broadcast across partitions with a K=1 ones matmul on the PE (the
     gpsimd partition_broadcast ucode costs an ~8us library load), and
     multiplied into ctx^T.
  5. out[s, e] += ctx^T.T @ wd^T; each ctx stationary serves two eo
     columns per load; the 8 output quanta of chunk c interleave into
     chunk c+1's attention stream so the PE always has independent work
     while ScalarE (exp) catches up.  Partials stored fp16; host sums f32.
"""

import numpy as np

B, S, E = 1, 2048, 2048
H, KVH, D = 16, 4, 128
NCORES = 8
P = 128
FD = 512            # matmul moving free dim == one fp32 PSUM bank
NE = E // P         # 16 contraction tiles over the embedding dim
NSC = S // FD       # 4 sequence chunks
NST = S // P        # 16 sequence tiles
FLOC = 4 * P        # local fused qkv rows per core (2 q heads + k + v)
ROPE_BASE = 10000.0
# causally visible query sub-range start for diagonal sk tile o
DIAG_START = (0, 128, 256, 384)
LOG_BIAS = -float(np.log(1024.0))   # exp bias; cancels in normalization
NWARM = 14

LAST_RESULT = None
_BASS_CACHE = None


def _rope_tables():
    inv = 1.0 / (ROPE_BASE ** (np.arange(0, D, 2, dtype=np.float64) / D))
    t = np.arange(S, dtype=np.float64)
    freqs = np.outer(t, inv)
    emb = np.concatenate([freqs, freqs], axis=-1)  # [S, D]
    return np.cos(emb), np.sin(emb)


def _host_constants():
    cos, sin = _rope_tables()
    consts = {}
    consts["cosr"] = np.ascontiguousarray(cos.T.astype(np.float16))
    consts["sinr"] = np.ascontiguousarray(sin.T.astype(np.float16))
    # [128, 128] lower-triangle(r <= c); multiplies every diagonal tile's
    # leading 128 columns after exp
    r_idx = np.arange(P)[:, None]
    c_idx = np.arange(P)[None, :]
    consts["maskm"] = np.ascontiguousarray((r_idx <= c_idx).astype(np.float16))
    # rotate_half as a matmul: rot = M @ q (in [d, s] layout); pass M.T as lhsT
    M = np.zeros((P, P), np.float16)
    half = D // 2
    M[np.arange(half), np.arange(half) + half] = -1.0
    M[np.arange(half) + half, np.arange(half)] = 1.0
    consts["protT"] = np.ascontiguousarray(M.T)
    consts["ebias"] = np.full((P, 1), LOG_BIAS, np.float32)
    consts["ident"] = np.eye(P, dtype=np.float32)
    consts["ones"] = np.ones((P, 1), np.float16)
    consts["onb"] = np.ones((1, P), np.float16)
    return consts


def _build_bass():
    import concourse.mybir as mybir
    import concourse.tile as tile
    from concourse import bacc

    f32 = mybir.dt.float32
    f32r = mybir.dt.float32r
    f16 = mybir.dt.float16
    Exp = mybir.ActivationFunctionType.Exp

    nc = bacc.Bacc(None, target_bir_lowering=False, name="mha_tp8")
    # x^T as [eo, p, s]: one fully contiguous 512KB DMA per eo slice
    xTt = nc.dram_tensor("xTt", [NE, P, S], f16, kind="ExternalInput")
    wqkvT = nc.dram_tensor("wqkvT", [E, FLOC], f16, kind="ExternalInput")
    wdT = nc.dram_tensor("wdT", [2 * P, S], f16, kind="ExternalInput")
    cosr = nc.dram_tensor("cosr", [P, S], f16, kind="ExternalInput")
    sinr = nc.dram_tensor("sinr", [P, S], f16, kind="ExternalInput")
    maskm = nc.dram_tensor("maskm", [P, P], f16, kind="ExternalInput")
    protT = nc.dram_tensor("protT", [P, P], f16, kind="ExternalInput")
    ident = nc.dram_tensor("ident", [P, P], f32, kind="ExternalInput")
    ones = nc.dram_tensor("ones", [P, 1], f16, kind="ExternalInput")
    onb = nc.dram_tensor("onb", [1, P], f16, kind="ExternalInput")
    ebias = nc.dram_tensor("ebias", [P, 1], f32, kind="ExternalInput")
    # output tiled [c, st, eo, p, f]; host reassembles to [s, e]
    out = nc.dram_tensor("out", [NSC, 4, 4, P, FD], f16, kind="ExternalOutput")

    with tile.TileContext(nc) as tc:
        with tc.tile_pool(name="const", bufs=1) as const:
            x_sb = const.tile([P, NE, S], f16, name="x_sb")
            w_sb = const.tile([P, NE, FLOC], f16, name="w_sb")
            pr = const.tile([P, P], f16, name="pr")
            idn = const.tile([P, P], f32, name="idn")
            on = const.tile([P, 1], f16, name="on")
            ob = const.tile([1, P], f16, name="ob")
            eb = const.tile([P, 1], f32, name="eb")
            warm = const.tile([P, FD], f16, name="warm")

            cq = const.tile([P, S], f16, name="cq")
            sq_t = const.tile([P, S], f16, name="sq_t")
            mk = const.tile([P, P], f16, name="mk")
            wd_sb = const.tile([P, 2, S], f16, name="wd_sb")

            qr = const.tile([P, 2, S], f16, name="qr")
            kr = const.tile([P, S], f16, name="kr")
            vT = const.tile([P, S], f32, name="vT")
            vn = const.tile([P, NST, P], f16, name="vn")

            # ---- Phase A: fused QKV projection + RoPE + v transpose ----
            with tc.tile_pool(name="ps_qkv", bufs=1, space="PSUM") as pqkv, \
                 tc.tile_pool(name="ps_rot", bufs=2, space="PSUM") as prot_p, \
                 tc.tile_pool(name="ps_vt", bufs=2, space="PSUM") as pvt, \
                 tc.tile_pool(name="rtmp", bufs=3) as rtmp:
                # HAM warmup: PE busy-work with no DMA dependency
                nc.vector.memset(warm, 0.0)
                for i in range(NWARM):
                    wp = prot_p.tile([P, FD], f32, tag="rot", name=f"warm_{i}")
                    nc.tensor.matmul(wp, lhsT=warm[:, :P], rhs=warm,
                                     start=True, stop=True)
                # x slices on the sync ring, weights on the gpsimd ring,
                # tables + dense weights on the scalar ring
                for eo in range(NE):
                    nc.sync.dma_start(x_sb[:, eo, :], xTt[eo])
                    nc.gpsimd.dma_start(
                        w_sb[:, eo, :], wqkvT[eo * P:(eo + 1) * P, :])
                nc.sync.dma_start(pr, protT[:, :])
                nc.sync.dma_start(idn, ident[:, :])
                nc.sync.dma_start(on, ones[:, :])
                nc.sync.dma_start(ob, onb[:, :])
                nc.sync.dma_start(eb, ebias[:, :])
                nc.scalar.dma_start(cq, cosr[:, :])
                nc.scalar.dma_start(sq_t, sinr[:, :])
                nc.scalar.dma_start(mk, maskm[:, :])
                nc.scalar.dma_start(wd_sb, wdT.rearrange("(h p) e -> p h e", p=P))

                def rope_pass(f, sc, pt_):
                    ssl = slice(sc * FD, (sc + 1) * FD)
                    dst = qr[:, f, ssl] if f < 2 else kr[:, ssl]
                    qt = rtmp.tile([P, FD], f16, tag="qt", name=f"qt_{sc}_{f}")
                    nc.scalar.copy(qt, pt_)
                    rp = prot_p.tile([P, FD], f32, tag="rot", name=f"rot_{sc}_{f}")
                    nc.tensor.matmul(rp, lhsT=pr, rhs=qt, start=True, stop=True)
                    tt = rtmp.tile([P, FD], f16, tag="tt", name=f"tt_{sc}_{f}")
                    nc.vector.tensor_mul(tt, rp, sq_t[:, ssl])
                    nc.vector.tensor_mul(dst, qt, cq[:, ssl])
                    nc.vector.tensor_add(dst, dst, tt)

                # f-outer passes: k, v, q0, q1; each weight tile loaded once
                # and streamed over the four chunks
                for f in (2, 3, 0, 1):
                    psums = [
                        pqkv.tile([P, FD], f32, tag=f"qkv{sc}", name=f"ps_{f}_{sc}")
                        for sc in range(NSC)
                    ]
                    for eo in range(NE):
                        with tc.tile_critical():
                            for sc in range(NSC):
                                mi = nc.tensor.matmul(
                                    psums[sc],
                                    lhsT=w_sb[:, eo, f * P:(f + 1) * P],
                                    rhs=x_sb[:, eo, sc * FD:(sc + 1) * FD],
                                    start=(eo == 0),
                                    stop=(eo == NE - 1),
                                )
                                if sc:
                                    mi.ins.ldweights = False
                    for sc in range(NSC):
                        if f == 3:
                            ssl = slice(sc * FD, (sc + 1) * FD)
                            nc.scalar.copy(vT[:, ssl], psums[sc])
                            for jj in range(4):
                                j = 4 * sc + jj
                                vp = pvt.tile([P, P], f32, tag="vt", name=f"vt_{j}")
                                nc.tensor.transpose(
                                    vp, vT[:, j * P:(j + 1) * P], idn)
                                nc.scalar.copy(vn[:, j, :], vp)
                        else:
                            rope_pass(f, sc, psums[sc])

            # ---- Phase B: attention + dense, per 512-query chunk ----
            with tc.tile_pool(name="ps_s", bufs=2, space="PSUM") as ps_s, \
                 tc.tile_pool(name="ps_ctx", bufs=2, space="PSUM") as ps_ctx, \
                 tc.tile_pool(name="ps_o", bufs=2, space="PSUM") as ps_o, \
                 tc.tile_pool(name="pt_p", bufs=3) as ptp, \
                 tc.tile_pool(name="acc_p", bufs=3) as accp, \
                 tc.tile_pool(name="rb_p", bufs=3) as rbp, \
                 tc.tile_pool(name="ctx_p", bufs=3) as ctxp, \
                 tc.tile_pool(name="out_p", bufs=4) as outp:
                all_csb = {}

                def dense_quantum(c, st, ep):
                    eos = (2 * ep, 2 * ep + 1)
                    ops = [
                        ps_o.tile([P, FD], f32, tag="o", name=f"o_{c}_{st}_{eo}")
                        for eo in eos
                    ]
                    for h in range(2):
                        with tc.tile_critical():
                            for i, eo in enumerate(eos):
                                mi = nc.tensor.matmul(
                                    ops[i],
                                    lhsT=all_csb[(c, h)][:, st * P:(st + 1) * P],
                                    rhs=wd_sb[:, h, eo * FD:(eo + 1) * FD],
                                    start=(h == 0), stop=(h == 1),
                                )
                                if i:
                                    mi.ins.ldweights = False
                    for i, eo in enumerate(eos):
                        ot = outp.tile([P, FD], f16, tag="ot", name=f"ot_{c}_{st}_{eo}")
                        # ScalarE is exp-bound in this phase: 1 release copy
                        # in 4 goes there, the rest to DVE
                        if (2 * st + ep) % 4 == 0:
                            nc.scalar.copy(ot, ops[i])
                        else:
                            nc.vector.tensor_copy(ot, ops[i])
                        nc.sync.dma_start(out[c, st, eo], ot)

                def dense_quanta(c):
                    if c < 0:
                        return iter(())
                    return iter([(c, st, ep) for st in range(4) for ep in range(2)])

                def emit_attn(c, dq):
                    qbase = c * FD
                    nj = 4 * c + 4
                    two_chain = c >= 1
                    ctxps = [
                        ps_ctx.tile([P, FD], f32, tag="ctx", name=f"ctx_{c}_{h}")
                        for h in range(2)
                    ]
                    accs = {}
                    for h in range(2):
                        acc_a = accp.tile([P, FD], f16, tag=f"acca{h}", name=f"acca_{c}_{h}")
                        acc_b = (
                            accp.tile([P, FD], f16, tag=f"accb{h}", name=f"accb_{c}_{h}")
                            if two_chain else None
                        )
                        accs[h] = (acc_a, acc_b)
                    for j in range(nj):
                        o = j - 4 * c
                        so = DIAG_START[o] if o >= 0 else 0
                        n = FD - so
                        sp2 = ps_s.tile([P, 2, FD], f32, tag="s", name=f"s_{c}_{j}")
                        with tc.tile_critical():
                            for h in range(2):
                                mi = nc.tensor.matmul(
                                    sp2[:, h, :n],
                                    lhsT=kr[:, j * P:(j + 1) * P],
                                    rhs=qr[:, h, qbase + so: qbase + FD],
                                    start=True, stop=True,
                                )
                                if h:
                                    mi.ins.ldweights = False
                        pt2 = ptp.tile([P, 2, FD], f16, tag="pt", name=f"pt_{c}_{j}")
                        nc.scalar.activation(
                            pt2[:, :, :n], sp2[:, :, :n], Exp, bias=eb[:, :])
                        for h in range(2):
                            if o >= 0:
                                nc.vector.tensor_mul(
                                    pt2[:, h, :P], pt2[:, h, :P], mk)
                            acc_a, acc_b = accs[h]
                            acc = acc_b if (two_chain and j % 2) else acc_a
                            if j < (2 if two_chain else 1):
                                nc.vector.tensor_copy(acc, pt2[:, h, :])
                            else:
                                nc.vector.tensor_add(
                                    acc[:, so:], acc[:, so:], pt2[:, h, :n])
                        with tc.tile_critical():
                            for h in range(2):
                                mi = nc.tensor.matmul(
                                    ctxps[h][:, so:],
                                    lhsT=vn[:, j, :],
                                    rhs=pt2[:, h, :n],
                                    start=(j == 0), stop=(j == nj - 1),
                                )
                                if h:
                                    mi.ins.ldweights = False
                        for q in (next(dq, None),):
                            if q is not None:
                                dense_quantum(*q)
                    # softmax tails after the tile loop
                    for h in range(2):
                        acc_a, acc_b = accs[h]
                        rp_ = ps_s.tile([1, FD], f32, tag="s", name=f"r_{c}_{h}")
                        if two_chain:
                            nc.tensor.matmul(rp_, lhsT=on, rhs=acc_a, start=True, stop=False)
                            nc.tensor.matmul(rp_, lhsT=on, rhs=acc_b, start=False, stop=True)
                        else:
                            nc.tensor.matmul(rp_, lhsT=on, rhs=acc_a, start=True, stop=True)
                        rec = rbp.tile([1, FD], f32, tag="rec", name=f"rec_{c}_{h}")
                        nc.vector.reciprocal_approx_fast(rec, rp_)
                        rec16 = rbp.tile([1, FD], f16, tag="rec16", name=f"rec16_{c}_{h}")
                        nc.vector.tensor_copy(rec16, rec)
                        # partition-broadcast of 1/denom via a K=1 matmul
                        rbps = ps_s.tile([P, FD], f32, tag="s", name=f"rb_{c}_{h}")
                        nc.tensor.matmul(rbps, lhsT=ob, rhs=rec16,
                                         start=True, stop=True)
                        rbs = rbp.tile([P, FD], f32, tag="rb", name=f"rb_{c}_{h}")
                        nc.scalar.copy(rbs, rbps)
                        ct = ctxp.tile([P, FD], f16, tag=f"ctx{h}", name=f"csb_{c}_{h}")
                        nc.vector.tensor_mul(ct, ctxps[h], rbs)
                        all_csb[(c, h)] = ct
                    for q in dq:
                        dense_quantum(*q)

                emit_attn(0, dense_quanta(-1))
                emit_attn(1, dense_quanta(0))
                emit_attn(2, dense_quanta(1))
                emit_attn(3, dense_quanta(2))
                for q in dense_quanta(3):
                    dense_quantum(*q)
    nc.compile()
    return nc


def make_in_maps(x, w_qkv, w_dense):
    x = np.asarray(x, np.float32).reshape(S, E)
    w_qkv = np.asarray(w_qkv, np.float32)
    w_dense = np.asarray(w_dense, np.float32)
    # x^T tiled to [eo, p, s]: contiguous per-eo slices
    xTt = np.ascontiguousarray(x.T.reshape(NE, P, S)).astype(np.float16)
    consts = _host_constants()
    in_maps = []
    scale = np.float32(1.0 / np.sqrt(D))
    for d in range(NCORES):
        g = d // 2
        wq = w_qkv[2 * d * P:(2 * d + 2) * P] * scale
        wk = w_qkv[H * D + g * P: H * D + (g + 1) * P]
        wv = w_qkv[H * D + KVH * D + g * P: H * D + KVH * D + (g + 1) * P]
        wqkvT_d = np.ascontiguousarray(
            np.concatenate([wq, wk, wv], 0).T
        ).astype(np.float16)
        wdT_d = np.ascontiguousarray(
            w_dense[:, 2 * d * P:(2 * d + 2) * P].T
        ).astype(np.float16)
        m = {"xTt": xTt, "wqkvT": wqkvT_d, "wdT": wdT_d}
        m.update(consts)
        in_maps.append(m)
    return in_maps


def kernel(x, w_qkv, w_dense):
    global LAST_RESULT, _BASS_CACHE
    from concourse.bass_utils import run_bass_kernel_spmd

    in_maps = make_in_maps(x, w_qkv, w_dense)
    if _BASS_CACHE is None:
        _BASS_CACHE = _build_bass()
    res = run_bass_kernel_spmd(_BASS_CACHE, in_maps, core_ids=list(range(NCORES)))
    LAST_RESULT = res
    # sum partials over cores, then untile [c, st, eo, p, f] -> [s, e]
    acc = np.zeros((NSC, 4, 4, P, FD), np.float32)
    for r in res.results:
        acc += r["out"]
    full = acc.transpose(0, 1, 3, 2, 4).reshape(S, E)
    return np.ascontiguousarray(full).reshape(B, S, E)


# revision 16
# speedup vs baseline: 3.7271x; 3.7271x over previous
"""Tensor-parallel MultiHeadAttention (QKV + RoPE + GQA causal SDPA + dense)
for 8 Trainium2 NeuronCores.

Sharding (TP as in TPMultiHeadAttention): core d owns query heads {2d, 2d+1}
and the single kv head d//2 (kv heads replicated across core pairs), plus the
matching 256 columns of the dense projection. Each core produces a full-shape
partial output; the all-reduce is a host-side sum over the 8 partials.

All matmul feeds are fp16 (full PE rate + fast weight load).  The PE's
per-matmul weight reload (~115ns, serial with the stream) is amortized
wherever consecutive matmuls share a stationary operand: the second+
matmuls clear `InstMatmult.ldweights` and are pinned adjacent to their
loader inside a tile_critical block.

Per-core pipeline:
  0. ~14 dummy warmup matmuls on a memset scratch tile flip the PE's HAM
     clock gate to full speed while the first DMAs stream in.
  1. x^T loaded fully resident (8MB fp16); qkv^T = W @ x^T computed in
     f-outer passes (k, v, q0, q1): each weight tile is loaded once and
     streams the four 512-query chunks (ldweights amortized 4x).
  2. RoPE on q,k via a permutation matmul (rotate_half) + DVE combine;
     softmax scale folded into the q-side weights.  v transposed via PE.
  3. S^T[sk, sq] per 128-row sk tile: both heads' scores go into one
     [P, 2, 512] two-bank PSUM pair (one kr weight load, one fused exp).
     Diagonal tiles cover only their visible query range (N in
     {512, 384, 256, 128}); exp(S^T - ln 1024) on ScalarE (bias keeps
     fp16 exp sums small and cancels in normalization); the leading
     128-wide partial triangle is masked multiplicatively after exp.
  4. ctx^T[d, sq] += v_nat.T @ P^T, both heads per vn weight load.
     Denominators: fp16 P^T tiles summed on DVE (2x packed) in two
     parallel chains, column-summed by a ones matmul, reciprocal'd, then
     broadcast across partitions with a K=1 ones matmul on the PE (the
     gpsimd partition_broadcast ucode costs an ~8us library load), and
     multiplied into ctx^T.
  5. out[s, e] += ctx^T.T @ wd^T; each ctx stationary serves two eo
     columns per load; the 8 output quanta of chunk c interleave into
     chunk c+1's attention stream so the PE always has independent work
     while ScalarE (exp) catches up.  Partials stored fp16; host sums f32.
"""

import numpy as np

B, S, E = 1, 2048, 2048
H, KVH, D = 16, 4, 128
NCORES = 8
P = 128
FD = 512            # matmul moving free dim == one fp32 PSUM bank
NE = E // P         # 16 contraction tiles over the embedding dim
NSC = S // FD       # 4 sequence chunks
NST = S // P        # 16 sequence tiles
FLOC = 4 * P        # local fused qkv rows per core (2 q heads + k + v)
ROPE_BASE = 10000.0
# causally visible query sub-range start for diagonal sk tile o
DIAG_START = (0, 128, 256, 384)
LOG_BIAS = -float(np.log(1024.0))   # exp bias; cancels in normalization
NWARM = 14

LAST_RESULT = None
_BASS_CACHE = None


def _rope_tables():
    inv = 1.0 / (ROPE_BASE ** (np.arange(0, D, 2, dtype=np.float64) / D))
    t = np.arange(S, dtype=np.float64)
    freqs = np.outer(t, inv)
    emb = np.concatenate([freqs, freqs], axis=-1)  # [S, D]
    return np.cos(emb), np.sin(emb)


def _host_constants():
    cos, sin = _rope_tables()
    consts = {}
    consts["cosr"] = np.ascontiguousarray(cos.T.astype(np.float16))
    consts["sinr"] = np.ascontiguousarray(sin.T.astype(np.float16))
    # [128, 128] lower-triangle(r <= c); multiplies every diagonal tile's
    # leading 128 columns after exp
    r_idx = np.arange(P)[:, None]
    c_idx = np.arange(P)[None, :]
    consts["maskm"] = np.ascontiguousarray((r_idx <= c_idx).astype(np.float16))
    # rotate_half as a matmul: rot = M @ q (in [d, s] layout); pass M.T as lhsT
    M = np.zeros((P, P), np.float16)
    half = D // 2
    M[np.arange(half), np.arange(half) + half] = -1.0
    M[np.arange(half) + half, np.arange(half)] = 1.0
    consts["protT"] = np.ascontiguousarray(M.T)
    consts["ebias"] = np.full((P, 1), LOG_BIAS, np.float32)
    consts["ident"] = np.eye(P, dtype=np.float32)
    consts["ones"] = np.ones((P, 1), np.float16)
    consts["onb"] = np.ones((1, P), np.float16)
    return consts


def _dedup_ldweights(nc):
    """Post-schedule pass: delete InstLdweights whose weights (and load
    mode) are already resident in the PE array, i.e. consecutive matmuls
    sharing a stationary operand only pay one weight load.  Operates on
    the final scheduled order, so it is safe against scheduler
    interleaving; any waits on a deleted load move to the next PE
    instruction (loads carrying semaphore updates are kept)."""
    import concourse.mybir as mybir

    MAX_WAITS = 1   # TPB matmult sync-wait slot limit
    removed = 0
    for blk in nc.main_func.blocks:
        insts = list(blk.instructions)
        loaded = None
        out = []
        for idx, inst in enumerate(insts):
            is_pe = getattr(inst, "engine", None) == mybir.EngineType.PE
            if is_pe and type(inst).__name__ == "InstLdweights":
                sig = (
                    str(inst.ins[0]),
                    str(getattr(inst, "is_transpose", None)),
                    str(getattr(inst, "perf_mode", None)),
                )
                si = inst.sync_info
                has_upd = si is not None and len(si.on_update) > 0
                nwait = 0 if si is None else len(si.on_wait)
                if sig == loaded and not has_upd:
                    # find the next PE instruction to absorb this load's waits
                    nxt = None
                    if nwait:
                        for later in insts[idx + 1:]:
                            if getattr(later, "engine", None) == mybir.EngineType.PE:
                                nxt = later
                                break
                    if nwait == 0 or (
                        nxt is not None
                        and (0 if nxt.sync_info is None
                             else len(nxt.sync_info.on_wait)) + nwait <= MAX_WAITS
                    ):
                        if nwait:
                            nsi = nxt.sync_info
                            if nsi is None:
                                nxt.sync_info = mybir.SyncInfo(
                                    on_wait=list(si.on_wait), on_update=[])
                            else:
                                nsi.on_wait = list(nsi.on_wait) + list(si.on_wait)
                        removed += 1
                        continue
                loaded = sig
                out.append(inst)
                continue
            out.append(inst)
        blk.instructions[:] = out
    return removed


def _build_bass():
    import concourse.mybir as mybir
    import concourse.tile as tile
    from concourse import bacc

    f32 = mybir.dt.float32
    f32r = mybir.dt.float32r
    f16 = mybir.dt.float16
    Exp = mybir.ActivationFunctionType.Exp

    nc = bacc.Bacc(None, target_bir_lowering=False, name="mha_tp8")
    # x^T as [eo, p, s]: one fully contiguous 512KB DMA per eo slice
    xTt = nc.dram_tensor("xTt", [NE, P, S], f16, kind="ExternalInput")
    wqkvT = nc.dram_tensor("wqkvT", [E, FLOC], f16, kind="ExternalInput")
    wdT = nc.dram_tensor("wdT", [2 * P, S], f16, kind="ExternalInput")
    cosr = nc.dram_tensor("cosr", [P, S], f16, kind="ExternalInput")
    sinr = nc.dram_tensor("sinr", [P, S], f16, kind="ExternalInput")
    maskm = nc.dram_tensor("maskm", [P, P], f16, kind="ExternalInput")
    protT = nc.dram_tensor("protT", [P, P], f16, kind="ExternalInput")
    ident = nc.dram_tensor("ident", [P, P], f32, kind="ExternalInput")
    ones = nc.dram_tensor("ones", [P, 1], f16, kind="ExternalInput")
    onb = nc.dram_tensor("onb", [1, P], f16, kind="ExternalInput")
    ebias = nc.dram_tensor("ebias", [P, 1], f32, kind="ExternalInput")
    # output tiled [c, st, eo, p, f]; host reassembles to [s, e]
    out = nc.dram_tensor("out", [NSC, 4, 4, P, FD], f16, kind="ExternalOutput")

    with tile.TileContext(nc) as tc:
        with tc.tile_pool(name="const", bufs=1) as const:
            x_sb = [
                const.tile([P, S], f16, name=f"x_sb{eo}") for eo in range(NE)
            ]
            w_sb = const.tile([P, NE, FLOC], f16, name="w_sb")
            pr = const.tile([P, P], f16, name="pr")
            idn = const.tile([P, P], f32, name="idn")
            on = const.tile([P, 1], f16, name="on")
            ob = const.tile([1, P], f16, name="ob")
            eb = const.tile([P, 1], f32, name="eb")
            warm = const.tile([P, FD], f16, name="warm")

            cq = const.tile([P, S], f16, name="cq")
            sq_t = const.tile([P, S], f16, name="sq_t")
            mk = const.tile([P, P], f16, name="mk")
            wd_sb = const.tile([P, 2, S], f16, name="wd_sb")

            qr = const.tile([P, 2, S], f16, name="qr")
            kr = const.tile([P, S], f16, name="kr")
            vT = const.tile([P, S], f32, name="vT")
            vn = const.tile([P, NST, P], f16, name="vn")

            # ---- Phase A: fused QKV projection + RoPE + v transpose ----
            with tc.tile_pool(name="ps_qkv", bufs=1, space="PSUM") as pqkv, \
                 tc.tile_pool(name="ps_rot", bufs=2, space="PSUM") as prot_p, \
                 tc.tile_pool(name="ps_vt", bufs=2, space="PSUM") as pvt, \
                 tc.tile_pool(name="rtmp", bufs=3) as rtmp:
                # HAM warmup: PE busy-work with no DMA dependency
                nc.vector.memset(warm, 0.0)
                for i in range(NWARM):
                    wp = prot_p.tile([P, FD], f32, tag="rot", name=f"warm_{i}")
                    nc.tensor.matmul(wp, lhsT=warm[:, :P], rhs=warm,
                                     start=True, stop=True)
                # x slices on the sync ring, weights on the gpsimd ring,
                # tables + dense weights on the scalar ring
                for eo in range(NE):
                    nc.sync.dma_start(x_sb[eo], xTt[eo])
                    nc.gpsimd.dma_start(
                        w_sb[:, eo, :], wqkvT[eo * P:(eo + 1) * P, :])
                nc.sync.dma_start(pr, protT[:, :])
                nc.sync.dma_start(idn, ident[:, :])
                nc.sync.dma_start(on, ones[:, :])
                nc.sync.dma_start(ob, onb[:, :])
                nc.sync.dma_start(eb, ebias[:, :])
                nc.scalar.dma_start(cq, cosr[:, :])
                nc.scalar.dma_start(sq_t, sinr[:, :])
                nc.scalar.dma_start(mk, maskm[:, :])
                nc.scalar.dma_start(wd_sb, wdT.rearrange("(h p) e -> p h e", p=P))

                def rope_pass(f, sc, pt_):
                    ssl = slice(sc * FD, (sc + 1) * FD)
                    dst = qr[:, f, ssl] if f < 2 else kr[:, ssl]
                    qt = rtmp.tile([P, FD], f16, tag="qt", name=f"qt_{sc}_{f}")
                    nc.scalar.copy(qt, pt_)
                    rp = prot_p.tile([P, FD], f32, tag="rot", name=f"rot_{sc}_{f}")
                    nc.tensor.matmul(rp, lhsT=pr, rhs=qt, start=True, stop=True)
                    tt = rtmp.tile([P, FD], f16, tag="tt", name=f"tt_{sc}_{f}")
                    nc.vector.tensor_mul(tt, rp, sq_t[:, ssl])
                    nc.vector.tensor_mul(dst, qt, cq[:, ssl])
                    nc.vector.tensor_add(dst, dst, tt)

                # f-outer passes: k, v, q0, q1; each weight tile loaded once
                # and streamed over the four chunks
                for f in (2, 3, 0, 1):
                    psums = [
                        pqkv.tile([P, FD], f32, tag=f"qkv{sc}", name=f"ps_{f}_{sc}")
                        for sc in range(NSC)
                    ]
                    for eo in range(NE):
                        for sc in range(NSC):
                            nc.tensor.matmul(
                                psums[sc],
                                lhsT=w_sb[:, eo, f * P:(f + 1) * P],
                                rhs=x_sb[eo][:, sc * FD:(sc + 1) * FD],
                                start=(eo == 0),
                                stop=(eo == NE - 1),
                            )
                    for sc in range(NSC):
                        if f == 3:
                            ssl = slice(sc * FD, (sc + 1) * FD)
                            nc.scalar.copy(vT[:, ssl], psums[sc])
                            for jj in range(4):
                                j = 4 * sc + jj
                                vp = pvt.tile([P, P], f32, tag="vt", name=f"vt_{j}")
                                nc.tensor.transpose(
                                    vp, vT[:, j * P:(j + 1) * P], idn)
                                nc.scalar.copy(vn[:, j, :], vp)
                        else:
                            rope_pass(f, sc, psums[sc])

            # ---- Phase B: attention + dense, per 512-query chunk ----
            with tc.tile_pool(name="ps_s", bufs=2, space="PSUM") as ps_s, \
                 tc.tile_pool(name="ps_ctx", bufs=2, space="PSUM") as ps_ctx, \
                 tc.tile_pool(name="ps_o", bufs=2, space="PSUM") as ps_o, \
                 tc.tile_pool(name="pt_p", bufs=3) as ptp, \
                 tc.tile_pool(name="acc_p", bufs=3) as accp, \
                 tc.tile_pool(name="rb_p", bufs=3) as rbp, \
                 tc.tile_pool(name="ctx_p", bufs=3) as ctxp, \
                 tc.tile_pool(name="out_p", bufs=4) as outp:
                all_csb = {}

                def dense_quantum(c, st, ep):
                    eos = (2 * ep, 2 * ep + 1)
                    ops = [
                        ps_o.tile([P, FD], f32, tag="o", name=f"o_{c}_{st}_{eo}")
                        for eo in eos
                    ]
                    for h in range(2):
                        for i, eo in enumerate(eos):
                            nc.tensor.matmul(
                                ops[i],
                                lhsT=all_csb[(c, h)][:, st * P:(st + 1) * P],
                                rhs=wd_sb[:, h, eo * FD:(eo + 1) * FD],
                                start=(h == 0), stop=(h == 1),
                            )
                    for i, eo in enumerate(eos):
                        ot = outp.tile([P, FD], f16, tag="ot", name=f"ot_{c}_{st}_{eo}")
                        # ScalarE is exp-bound in this phase: 1 release copy
                        # in 4 goes there, the rest to DVE
                        if (2 * st + ep) % 4 == 0:
                            nc.scalar.copy(ot, ops[i])
                        else:
                            nc.vector.tensor_copy(ot, ops[i])
                        nc.sync.dma_start(out[c, st, eo], ot)

                def dense_quanta(c):
                    if c < 0:
                        return iter(())
                    return iter([(c, st, ep) for st in range(4) for ep in range(2)])

                def emit_attn(c, dq):
                    qbase = c * FD
                    nj = 4 * c + 4
                    two_chain = c >= 1
                    ctxps = [
                        ps_ctx.tile([P, FD], f32, tag="ctx", name=f"ctx_{c}_{h}")
                        for h in range(2)
                    ]
                    accs = {}
                    for h in range(2):
                        acc_a = accp.tile([P, FD], f16, tag=f"acca{h}", name=f"acca_{c}_{h}")
                        acc_b = (
                            accp.tile([P, FD], f16, tag=f"accb{h}", name=f"accb_{c}_{h}")
                            if two_chain else None
                        )
                        accs[h] = (acc_a, acc_b)
                    for j in range(nj):
                        o = j - 4 * c
                        so = DIAG_START[o] if o >= 0 else 0
                        n = FD - so
                        sp2 = ps_s.tile([P, 2, FD], f32, tag="s", name=f"s_{c}_{j}")
                        for h in range(2):
                            nc.tensor.matmul(
                                sp2[:, h, :n],
                                lhsT=kr[:, j * P:(j + 1) * P],
                                rhs=qr[:, h, qbase + so: qbase + FD],
                                start=True, stop=True,
                            )
                        pt2 = ptp.tile([P, 2, FD], f16, tag="pt", name=f"pt_{c}_{j}")
                        nc.scalar.activation(
                            pt2[:, :, :n], sp2[:, :, :n], Exp, bias=eb[:, :])
                        for h in range(2):
                            if o >= 0:
                                nc.vector.tensor_mul(
                                    pt2[:, h, :P], pt2[:, h, :P], mk)
                            acc_a, acc_b = accs[h]
                            acc = acc_b if (two_chain and j % 2) else acc_a
                            if j < (2 if two_chain else 1):
                                nc.vector.tensor_copy(acc, pt2[:, h, :])
                            else:
                                nc.vector.tensor_add(
                                    acc[:, so:], acc[:, so:], pt2[:, h, :n])
                        for h in range(2):
                            nc.tensor.matmul(
                                ctxps[h][:, so:],
                                lhsT=vn[:, j, :],
                                rhs=pt2[:, h, :n],
                                start=(j == 0), stop=(j == nj - 1),
                            )
                        for q in (next(dq, None),):
                            if q is not None:
                                dense_quantum(*q)
                    # softmax tails after the tile loop
                    for h in range(2):
                        acc_a, acc_b = accs[h]
                        rp_ = ps_s.tile([1, FD], f32, tag="s", name=f"r_{c}_{h}")
                        if two_chain:
                            nc.tensor.matmul(rp_, lhsT=on, rhs=acc_a, start=True, stop=False)
                            nc.tensor.matmul(rp_, lhsT=on, rhs=acc_b, start=False, stop=True)
                        else:
                            nc.tensor.matmul(rp_, lhsT=on, rhs=acc_a, start=True, stop=True)
                        rec = rbp.tile([1, FD], f32, tag="rec", name=f"rec_{c}_{h}")
                        nc.vector.reciprocal_approx_fast(rec, rp_)
                        rec16 = rbp.tile([1, FD], f16, tag="rec16", name=f"rec16_{c}_{h}")
                        nc.vector.tensor_copy(rec16, rec)
                        # partition-broadcast of 1/denom via a K=1 matmul
                        rbps = ps_s.tile([P, FD], f32, tag="s", name=f"rb_{c}_{h}")
                        nc.tensor.matmul(rbps, lhsT=ob, rhs=rec16,
                                         start=True, stop=True)
                        rbs = rbp.tile([P, FD], f32, tag="rb", name=f"rb_{c}_{h}")
                        nc.scalar.copy(rbs, rbps)
                        ct = ctxp.tile([P, FD], f16, tag=f"ctx{h}", name=f"csb_{c}_{h}")
                        nc.vector.tensor_mul(ct, ctxps[h], rbs)
                        all_csb[(c, h)] = ct
                    for q in dq:
                        dense_quantum(*q)

                emit_attn(0, dense_quanta(-1))
                emit_attn(1, dense_quanta(0))
                emit_attn(2, dense_quanta(1))
                emit_attn(3, dense_quanta(2))
                for q in dense_quanta(3):
                    dense_quantum(*q)
    nc.compile()
    return nc


def make_in_maps(x, w_qkv, w_dense):
    x = np.asarray(x, np.float32).reshape(S, E)
    w_qkv = np.asarray(w_qkv, np.float32)
    w_dense = np.asarray(w_dense, np.float32)
    # x^T tiled to [eo, p, s]: contiguous per-eo slices
    xTt = np.ascontiguousarray(x.T.reshape(NE, P, S)).astype(np.float16)
    consts = _host_constants()
    in_maps = []
    scale = np.float32(1.0 / np.sqrt(D))
    for d in range(NCORES):
        g = d // 2
        wq = w_qkv[2 * d * P:(2 * d + 2) * P] * scale
        wk = w_qkv[H * D + g * P: H * D + (g + 1) * P]
        wv = w_qkv[H * D + KVH * D + g * P: H * D + KVH * D + (g + 1) * P]
        wqkvT_d = np.ascontiguousarray(
            np.concatenate([wq, wk, wv], 0).T
        ).astype(np.float16)
        wdT_d = np.ascontiguousarray(
            w_dense[:, 2 * d * P:(2 * d + 2) * P].T
        ).astype(np.float16)
        m = {"xTt": xTt, "wqkvT": wqkvT_d, "wdT": wdT_d}
        m.update(consts)
        in_maps.append(m)
    return in_maps


def kernel(x, w_qkv, w_dense):
    global LAST_RESULT, _BASS_CACHE
    from concourse.bass_utils import run_bass_kernel_spmd

    in_maps = make_in_maps(x, w_qkv, w_dense)
    if _BASS_CACHE is None:
        _BASS_CACHE = _build_bass()
        import os as _os
        if _os.environ.get("KERNEL_NO_DEDUP", "") != "1":
            _dedup_ldweights(_BASS_CACHE)
    res = run_bass_kernel_spmd(_BASS_CACHE, in_maps, core_ids=list(range(NCORES)))
    LAST_RESULT = res
    # sum partials over cores, then untile [c, st, eo, p, f] -> [s, e]
    acc = np.zeros((NSC, 4, 4, P, FD), np.float32)
    for r in res.results:
        acc += r["out"]
    full = acc.transpose(0, 1, 3, 2, 4).reshape(S, E)
    return np.ascontiguousarray(full).reshape(B, S, E)
